# revision 1
# baseline (speedup 1.0000x reference)
"""Trainium2 Bass kernel for nn_AutoregressiveRegression (LSTM warmup + autoregressive decode).

Problem (per reference):
  B=512, T=128, F=4, U=1024, out_steps=32
  - warmup: LSTM over x[:, t, :] for t in 0..T-1 (h0=c0=0)
  - pred0 = h @ dense_w + dense_b
  - decode: 31 more LSTM steps feeding pred back as the input
  - output: [B, out_steps, 4]

Strategy (8 NeuronCores, data-parallel over batch, 64 rows/core):
  - Everything resident in SBUF, loaded by ONE DMA (single packed constant
    tensor); zero steady-state DMA, zero collectives.
  - Per step, z = [x_t; 1] @ [kernel; bias] + h @ rec_kernel computed as
    PSUM-accumulated matmuls with the *weights streaming* (moving operand) and
    h^T/x^T as the stationary operand (M=64).
  - float32r matmuls: 1 cycle/row at N=512 (4x faster than fp32; measured
    HW rel err vs fp32 reference ~3e-4).
  - Weight columns are gate-interleaved host-side: for each 128-unit slice j,
    columns are [i_j | f_j | o_j | g_j], so gate math for slice j is local to
    one [64,512] PSUM tile. (This walrus build only accepts matmul PSUM
    destinations starting at partition 0, so per-step work is 8 such chunks
    at M=64; per-step PE time is set by streaming the 16 MiB weight slice
    through the array: ~15.5 us/step measured, ~2.46 ms total.)
  - h is re-transposed each step on the PE (transpose-mode matmul) to produce
    the next step's stationary operands.
  - decode feedback pred^T = dense_w^T @ h^T computed directly in transposed
    form; bias folded in as an extra "ones" row of the stationary.
"""

import os
from contextlib import ExitStack

import numpy as np

B_FULL = 512
T_WARM = 128
N_CORES = 8
B_LOC = B_FULL // N_CORES  # 64
U = 1024
NF = 4

# packed constant-tile column layout (fp32 elements per partition)
_WR0 = 0                      # rec_kernel, chunk-major: [128, 8*4096]
_KB0 = _WR0 + 8 * 4 * U       # kernel+bias rows 0:5: [5, 4096]
_DW0 = _KB0 + 4 * U           # dense_w chunk-major: [128, 32]
_ID0 = _DW0 + 32              # identity: [128, 128]
_XT0 = _ID0 + 128             # x^T + ones row: [5, T*b]
_IA0 = _XT0 + T_WARM * B_LOC  # decode io block: [5, S*b] (row 4 = ones)


def _ia_cols(S):
    return S * B_LOC


def _db_col(S):
    return _IA0 + _ia_cols(S)  # dense_b: [4, 1]


def _cst_cols(S):
    return _db_col(S) + 1


def _build_program(S, reps=1):
    """Build the per-core Bass program (identical on all cores; data differs).

    reps > 1 wraps the whole computation (including load DMAs) in a hardware
    For_i loop — used only for timing (slope over reps isolates on-device
    exec time from the ~80 ms axon RPC noise)."""
    import concourse.mybir as mybir
    import concourse.tile as tile
    from concourse import bacc

    F32 = mybir.dt.float32
    F32R = mybir.dt.float32r
    AF = mybir.ActivationFunctionType

    T = T_WARM
    b = B_LOC
    NSTEPS = T + S - 1  # 159 recurrent steps

    nc = bacc.Bacc("TRN2", target_bir_lowering=False, debug=False)

    cst_d = nc.dram_tensor("cst", [128, _cst_cols(S)], F32R,
                           kind="ExternalInput").ap()
    outp_d = nc.dram_tensor("outp", [4, S * b], F32R, kind="ExternalOutput").ap()

    def hT_col(k):
        # hT tile columns: pair q holds [hT_{2q} | hT_{2q+1}] at 128q / 128q+64
        return 128 * (k // 2) + 64 * (k % 2)

    with tile.TileContext(nc) as tc, ExitStack() as ctx:
        singles = ctx.enter_context(tc.tile_pool(name="singles", bufs=1))
        hTpool = ctx.enter_context(tc.tile_pool(name="hTpool", bufs=2))
        hpool = ctx.enter_context(tc.tile_pool(name="hpool", bufs=2))
        gpool = ctx.enter_context(tc.tile_pool(name="gpool", bufs=2))
        zpool = ctx.enter_context(tc.tile_pool(name="zpool", bufs=5, space="PSUM"))
        tppool = ctx.enter_context(tc.tile_pool(name="tppool", bufs=2, space="PSUM"))
        ptpool = ctx.enter_context(tc.tile_pool(name="ptpool", bufs=1, space="PSUM"))

        rep_ctx = tc.For_i(0, reps, 1) if reps > 1 else None
        if rep_ctx is not None:
            rep_ctx.__enter__()

        cst = singles.tile([128, _cst_cols(S)], F32R, tag="cst")
        nc.sync.dma_start(out=cst, in_=cst_d)

        wr_sb = [cst[:, _WR0 + k * 4 * U : _WR0 + (k + 1) * 4 * U]
                 for k in range(8)]
        kb_sb = cst[0:5, _KB0 : _KB0 + 4 * U]
        dw_sb = cst[:, _DW0 : _DW0 + 32]
        ident64 = cst[0:64, _ID0 : _ID0 + 64]
        xt_sb = cst[0:5, _XT0 : _XT0 + T * b]
        in_all = cst[0:5, _IA0 : _IA0 + S * b]
        # tensor_scalar_add needs an F32 scalar operand; gpsimd DMA casts
        # the F32R view into a tiny F32 tile
        db_sb = singles.tile([4, 1], F32, tag="db")
        nc.gpsimd.dma_start(out=db_sb, in_=cst_d[0:4, _db_col(S) : _db_col(S) + 1])

        c_sb = singles.tile([64, 8 * 128], F32, tag="c")
        nc.vector.memset(c_sb, 0.0)

        hT_prev = None
        for t in range(NSTEPS):
            warm = t < T
            if warm:
                in_stat = xt_sb[:, t * b : (t + 1) * b]
            else:
                dprev = t - T
                in_stat = in_all[:, dprev * b : (dprev + 1) * b]

            hT_cur = hTpool.tile([128, 512], F32R, tag="hT")
            h_cur = hpool.tile([64, 8 * 128], F32R, tag="h")
            for j in range(8):
                z = zpool.tile([64, 512], F32, tag="z")
                nA = 512 * j

                def mm(stat, mov, start, stop):
                    nc.tensor.matmul(z, stat, mov, start=start, stop=stop,
                                     skip_group_check=True)

                if t == 0:
                    # h = 0: input-chunk only
                    mm(in_stat, kb_sb[:, nA : nA + 512], True, True)
                elif warm:
                    mm(in_stat, kb_sb[:, nA : nA + 512], True, False)
                    for k in range(8):
                        mm(hT_prev[:, 64 * k : 64 * k + b],
                           wr_sb[k][:, nA : nA + 512], False, k == 7)
                else:
                    # decode: input chunk last (pred arrives latest)
                    for k in range(8):
                        mm(hT_prev[:, 64 * k : 64 * k + b],
                           wr_sb[k][:, nA : nA + 512], k == 0, False)
                    mm(in_stat, kb_sb[:, nA : nA + 512], False, True)

                # gate math; z cols: [i 0:128 | f 128:256 | o 256:384 | g 384:512]
                sfo = gpool.tile([64, 384], F32, tag="sfo")
                nc.scalar.activation(sfo, z[:, 0:384], AF.Sigmoid)
                gt = gpool.tile([64, 128], F32, tag="gt")
                nc.scalar.activation(gt, z[:, 384:512], AF.Tanh)
                t1 = gpool.tile([64, 128], F32, tag="t1")
                nc.vector.tensor_mul(t1, sfo[:, 0:128], gt)
                cj = c_sb[:, 128 * j : 128 * (j + 1)]
                nc.vector.tensor_mul(cj, sfo[:, 128:256], cj)
                nc.vector.tensor_add(cj, cj, t1)
                tct = gpool.tile([64, 128], F32, tag="tct")
                nc.scalar.activation(tct, cj, AF.Tanh)
                hj = h_cur[:, 128 * j : 128 * (j + 1)]
                nc.vector.tensor_mul(hj, sfo[:, 256:384], tct)

                tp = tppool.tile([128, 64], F32R, tag="tp")
                nc.tensor.transpose(tp, hj, ident64)
                nc.vector.tensor_copy(hT_cur[:, 64 * j : 64 * (j + 1)], tp)

            if t >= T - 1:
                d = t - (T - 1)
                pt = ptpool.tile([4, b], F32, tag="pt")
                for k in range(8):
                    nc.tensor.matmul(
                        pt, dw_sb[:, 4 * k : 4 * k + 4],
                        hT_cur[:, 64 * k : 64 * k + b],
                        start=(k == 0), stop=(k == 7),
                    )
                nc.vector.tensor_scalar_add(
                    in_all[0:4, d * b : (d + 1) * b], pt, db_sb
                )
            hT_prev = hT_cur

        nc.sync.dma_start(out=outp_d, in_=in_all[0:4, :])

        if rep_ctx is not None:
            rep_ctx.__exit__(None, None, None)

    nc.compile()  # bacc passes: wait-splitting (TRN2 allows 1 wait/inst), DCE
    return nc


def _prep_inputs(x, kern, rec_kernel, bias, dense_w, dense_b, S):
    """Host-side numpy prep: gate interleave, transposes, per-core shards."""
    T, b = T_WARM, B_LOC
    f32 = np.float32
    # interleaved column order: per 128-unit slice j -> [i_j, f_j, o_j, g_j]
    perm = np.concatenate(
        [g * U + np.arange(128 * j, 128 * (j + 1))
         for j in range(8) for g in (0, 1, 3, 2)]
    )
    C = _cst_cols(S)
    base = np.zeros((128, C), f32)
    base[:, _WR0 : _WR0 + 8 * 4 * U] = (
        rec_kernel[:, perm].reshape(8, 128, 4 * U).transpose(1, 0, 2)
        .reshape(128, 8 * 4 * U)
    )
    base[0:4, _KB0 : _KB0 + 4 * U] = kern[:, perm]
    base[4, _KB0 : _KB0 + 4 * U] = bias[perm]
    base[:, _DW0 : _DW0 + 32] = (
        dense_w.reshape(8, 128, NF).transpose(1, 0, 2).reshape(128, 32)
    )
    base[:, _ID0 : _ID0 + 128] = np.eye(128, dtype=f32)
    base[4, _IA0 : _IA0 + S * b] = 1.0  # decode ones row
    base[0:4, _db_col(S)] = dense_b

    in_maps = []
    for m in range(N_CORES):
        cst = base.copy()
        xs = x[m * b : (m + 1) * b].astype(f32)  # [b, T, F]
        xT = xs.transpose(2, 1, 0).reshape(NF, T * b)  # col index = t*b + b_idx
        cst[0:4, _XT0 : _XT0 + T * b] = xT
        cst[4, _XT0 : _XT0 + T * b] = 1.0
        in_maps.append({"cst": np.ascontiguousarray(cst)})
    return in_maps


def kernel(x, kernel, rec_kernel, bias, dense_w, dense_b, out_steps):
    from concourse import bass_utils

    S = int(out_steps)
    x = np.asarray(x, dtype=np.float32)
    nc = _build_program(S)
    in_maps = _prep_inputs(
        x, np.asarray(kernel, np.float32), np.asarray(rec_kernel, np.float32),
        np.asarray(bias, np.float32), np.asarray(dense_w, np.float32),
        np.asarray(dense_b, np.float32), S,
    )
    res = bass_utils.run_bass_kernel_spmd(
        nc, in_maps, core_ids=list(range(N_CORES)),
        trace=bool(int(os.environ.get("LSTM_KERNEL_TRACE", "0"))),
    )
    outs = []
    for m in range(N_CORES):
        o = res.results[m]["outp"]  # [4, S*b]
        outs.append(o.reshape(NF, S, B_LOC).transpose(2, 1, 0))  # [b, S, 4]
    return np.concatenate(outs, axis=0).astype(np.float32)  # [B, S, 4]

